# revision 52
# baseline (speedup 1.0000x reference)
"""Trainium2 Bass kernel for an 8-head GLU multi-head self-attention block.

Shapes (hardcoded from the problem spec):
  x [4, 2048, 1024], mask [4, 2048] (int32),
  W_q/W_k [1024, 2048], W_v [1024, 4096], W_o [2048, 2048],
  b_q/b_k [2048], b_v [4096], b_o [2048]  ->  out [4, 2048, 1024] f32.

Sharding: 8 cores = 4 batches x 2 query-halves. Each core computes K/V
projections for its full batch (duplicated within the pair - keeps the
program collective-free), Q projection + attention + output projection +
GLUs for its 1024-query half, all 8 heads.

Single merged pass: per head, the V/K/Q projections write straight into
SBUF (no DRAM spill), then attention for that head runs immediately;
GLU'd per-head outputs accumulate into resident SBUF tiles that phase C
consumes directly. x stays SBUF-resident (bf16 for V-proj, fp8 for K/Q
DoubleRow matmuls); W_o streams in while head 7's attention runs. The
only HBM traffic is the original inputs plus the output. All bias/mask
scaling variants (0.5x, 16x) are prepared on host so the kernel's const
loads are four single-descriptor strided DMAs.
"""

import sys
import numpy as np

for _p in ("/opt/trn_rl_repo", "/root/.axon_site/_ro/trn_rl_repo"):
    if _p not in sys.path:
        sys.path.insert(0, _p)

import ml_dtypes
import concourse.bass as bass
import concourse.mybir as mybir
import concourse.tile as tile
from concourse import bacc
from concourse.bass_utils import run_bass_kernel_spmd

F32 = mybir.dt.float32
F32R = mybir.dt.float32r
BF16 = mybir.dt.bfloat16
FP8E4 = mybir.dt.float8e4
DR = mybir.MatmulPerfMode.DoubleRow
AL = mybir.AluOpType
AF = mybir.ActivationFunctionType

N_CORES = 8
S = 2048          # sequence length
D = 1024          # d_model
H = 8             # heads
DK = 256          # per-head q/k dim
DV = 512          # per-head v dim (GLU-doubled)
DO = 2048         # output-projection dim (GLU-doubled)
QH = S // 2       # queries per core


def _bcast_ap(vec_ap, parts, offset, n):
    """AP reading vec[offset:offset+n] broadcast across `parts` partitions."""
    return bass.AP(tensor=vec_ap.tensor, offset=offset, ap=[[0, parts], [1, n]])


def _col_ap(vec_ap, cols):
    """AP viewing a [128*cols] vector as a [128, cols] column tile."""
    return bass.AP(tensor=vec_ap.tensor, offset=0, ap=[[1, 128], [128, cols]])


def _build():
    nc = bacc.Bacc("TRN2", target_bir_lowering=False, debug=False,
                   num_devices=N_CORES)

    xT = nc.dram_tensor("xT", [D, S], BF16, kind="ExternalInput").ap()
    xq8 = nc.dram_tensor("xq8", [D, QH], FP8E4, kind="ExternalInput").ap()
    wq = nc.dram_tensor("wq", [D, H * DK], FP8E4, kind="ExternalInput").ap()
    wk = nc.dram_tensor("wk", [D, H * DK], FP8E4, kind="ExternalInput").ap()
    wv = nc.dram_tensor("wv", [D, H * DV], BF16, kind="ExternalInput").ap()
    wo = nc.dram_tensor("wo", [H * DK, DO], BF16, kind="ExternalInput").ap()
    bq16 = nc.dram_tensor("bq16", [H * DK], F32, kind="ExternalInput").ap()
    bk16 = nc.dram_tensor("bk16", [H * DK], F32, kind="ExternalInput").ap()
    bv = nc.dram_tensor("bv", [H * DV], F32, kind="ExternalInput").ap()
    bvh = nc.dram_tensor("bvh", [H * DV], F32, kind="ExternalInput").ap()
    boh = nc.dram_tensor("boh", [DO], F32, kind="ExternalInput").ap()
    maskf = nc.dram_tensor("maskf", [S], F32, kind="ExternalInput").ap()
    maskh = nc.dram_tensor("maskh", [S], F32, kind="ExternalInput").ap()
    # out is [queries, d_model]: phase C computes G^T-stationary @ wo-moving
    # with queries on the PSUM partition dim, so no output transpose.
    out = nc.dram_tensor("out", [QH, D], F32, kind="ExternalOutput").ap()

    with tile.TileContext(nc) as tc:
        with tc.tile_pool(name="consts", bufs=1) as consts:
            # Single-descriptor strided loads on the (empty) scalar queue.
            mcol = consts.tile([128, S // 128], F32)
            nc.scalar.dma_start(out=mcol, in_=_col_ap(maskf, S // 128))
            mhalf = consts.tile([128, S // 128], F32)
            nc.scalar.dma_start(out=mhalf, in_=_col_ap(maskh, S // 128))
            # head-0 value-bias broadcast next (first V evac needs it early)
            bvc0 = consts.tile([128, DV], F32, name="bvc0")
            nc.scalar.dma_start(out=bvc0, in_=_bcast_ap(bv, 128, 0, DV))
            bq16_cols = consts.tile([128, H * DK // 128], F32)
            nc.scalar.dma_start(out=bq16_cols, in_=_col_ap(bq16, H * DK // 128))
            bk16_cols = consts.tile([128, H * DK // 128], F32)
            nc.scalar.dma_start(out=bk16_cols, in_=_col_ap(bk16, H * DK // 128))
            # Rows/columns of ones for the denominator broadcast matmuls.
            ones_f = consts.tile([1, 128], F32)
            nc.vector.memset(ones_f, 1.0)
            ones1 = consts.tile([1, 128], F32R)
            nc.vector.tensor_copy(ones1, ones_f)
            ones_c = consts.tile([128, 1], F32)
            nc.vector.memset(ones_c, 1.0)
            ones128 = consts.tile([128, 1], F32R)
            nc.vector.tensor_copy(ones128, ones_c)

            with tc.tile_pool(name="pa", bufs=2) as pa:
                ps_mm_cm = tc.tile_pool(name="ps_mm", bufs=3, space="PSUM")
                ps_ot_cm = tc.tile_pool(name="ps_ot", bufs=4, space="PSUM")
                ps_dn_cm = tc.tile_pool(name="ps_dn", bufs=1, space="PSUM")
                ps_mm = ps_mm_cm.__enter__()
                ps_ot = ps_ot_cm.__enter__()
                ps_dn = ps_dn_cm.__enter__()

                # Per-head GLU'd attention outputs, resident until phase C.
                Gch = [pa.tile([128, QH], BF16, tag="gch", bufs=16, name="gch")
                       for _ in range(H * DK // 128)]

                pending_tail = [None]
                wo5 = {}

                def attention(h, V_h, KT8h, QT8h):
                    for qc in range(QH // 512):
                        q0 = qc * 512
                        if pending_tail[0] is not None:
                            pending_tail[0]()
                            pending_tail[0] = None
                        ET = []
                        acc = None
                        for kt in range(S // 128):
                            st = ps_mm.tile([128, 512], F32, tag="mm")
                            nc.tensor.matmul(st, KT8h[:, :, kt * 128:(kt + 1) * 128],
                                             QT8h[:, :, q0:q0 + 512],
                                             start=True, stop=True, perf_mode=DR)
                            e = pa.tile([128, 512], BF16, tag="et", bufs=17)
                            nc.scalar.activation(e, st, AF.Exp, scale=0.0625)
                            ET.append(e)
                            # masked-exp running sum on DVE (ping-pong)
                            nacc = pa.tile([128, 512], F32R, tag="acc", bufs=2,
                                           name="acc")
                            if acc is None:
                                nc.vector.tensor_scalar(nacc, e, mcol[:, kt:kt + 1],
                                                        None, op0=AL.mult)
                            else:
                                nc.vector.scalar_tensor_tensor(
                                    nacc, e, mcol[:, kt:kt + 1], acc,
                                    op0=AL.mult, op1=AL.add)
                            acc = nacc
                        ots = [ps_ot.tile([128, 512], F32, tag="ot", name=f"ot{_i}")
                               for _i in range(4)]
                        for kt in range(S // 128):
                            for dvt in range(4):
                                nc.tensor.matmul(ots[dvt],
                                                 V_h[kt][:, dvt * 128:(dvt + 1) * 128],
                                                 ET[kt], start=(kt == 0),
                                                 stop=(kt == S // 128 - 1))
                        den = ps_dn.tile([1, 512], F32, tag="den")
                        nc.tensor.matmul(den, ones128, acc, start=True, stop=True)
                        dsb = pa.tile([1, 512], F32R, tag="dsb", bufs=2)
                        nc.vector.tensor_copy(dsb, den)
                        bcp = ps_dn.tile([128, 512], F32, tag="den")
                        nc.tensor.matmul(bcp, ones1, dsb, start=True, stop=True)
                        bc = pa.tile([128, 512], F32, tag="bc", bufs=2)
                        nc.vector.reciprocal_approx_fast(bc, bcp)

                        def _tail(h=h, q0=q0, ots=ots, bc=bc):
                            # all four ots reads first so the PSUM bank ring
                            # frees before the ACT-gated Gch writes; the GLU
                            # sigmoid identity's 0.5 rides the a-half op here
                            ans, gns = [], []
                            for c2 in range(2):
                                an = pa.tile([128, 512], F32, tag="an", bufs=2,
                                             name="an")
                                nc.vector.scalar_tensor_tensor(
                                    an, ots[c2], 0.5, bc,
                                    op0=AL.mult, op1=AL.mult)
                                gn = pa.tile([128, 512], F32, tag="gn", bufs=2,
                                             name="gn")
                                nc.vector.tensor_tensor(gn, ots[2 + c2], bc, AL.mult)
                                ans.append(an)
                                gns.append(gn)
                            for c2 in range(2):
                                tg = pa.tile([128, 512], F32, tag="tg", bufs=2,
                                             name="tg")
                                nc.scalar.activation(tg, gns[c2], AF.Tanh, scale=0.5)
                                nc.vector.scalar_tensor_tensor(
                                    Gch[2 * h + c2][:, q0:q0 + 512], tg, 1.0,
                                    ans[c2], op0=AL.add, op1=AL.mult)
                        pending_tail[0] = _tail

                # -------- merged projection + attention loop over heads --------
                px_cm = tc.tile_pool(name="px", bufs=2)
                px = px_cm.__enter__()

                # x resident: bf16 chunks (V-proj stationary) on sync, fp8
                # (K/Q DoubleRow moving) on gpsimd. Head-0 V weights go on
                # the scalar queue so they stream concurrently with xT.
                # Interleave xT-kc0 with head-0 V weights on sync so the first
                # V psum group can start ASAP; the scalar queue carries ONLY
                # the const loads (ACT owns the scalar engine once attention
                # starts — DMA descriptors there would head-of-line block exp).
                wvb_all = {}
                xT_ch = {}
                for d in range(D // 128):
                    tw = px.tile([128, DV], BF16, tag="wvb", bufs=16, name="wvb")
                    eng = [nc.gpsimd, nc.scalar][d % 2]
                    eng.dma_start(out=tw, in_=wv[d * 128:(d + 1) * 128, 0:DV])
                    wvb_all.setdefault(0, []).append(tw)
                    t = px.tile([128, 512], BF16, tag="xT_sb", bufs=16, name="xT_c")
                    nc.sync.dma_start(out=t, in_=xT[d * 128:(d + 1) * 128, 0:512])
                    xT_ch[(d, 0)] = t
                bo_ah = consts.tile([128, D], F32, name="bo_ah")
                bo_gh = consts.tile([128, D], F32, name="bo_gh")
                for d in range(D // 128):
                    t = px.tile([128, 512], BF16, tag="xT_sb", bufs=16,
                                name="xT_c")
                    nc.sync.dma_start(out=t, in_=xT[d * 128:(d + 1) * 128,
                                                   512:1024])
                    xT_ch[(d, 1)] = t
                for d in range(D // 128):
                    t = px.tile([128, 1024], BF16, tag="xT_sb2", bufs=8,
                                name="xT_c2")
                    nc.sync.dma_start(out=t, in_=xT[d * 128:(d + 1) * 128,
                                                   1024:2048])
                    xT_ch[(d, 2)] = t
                    xT_ch[(d, 3)] = t
                # fp8 x for the DoubleRow K matmuls is cast on-device from the
                # resident bf16 xT by the (idle until attention) ACT engine —
                # saves 2MB of HBM reads during the bandwidth-bound start.
                xp8_ch = {}
                for kc in range(S // 512):
                    for dp in range(D // 256):
                        t8 = px.tile([128, 2, 512], FP8E4, tag="xp8", bufs=16,
                                     name="xp8_c")
                        for half in range(2):
                            d = dp * 2 + half
                            if kc < 2:
                                src = xT_ch[(d, kc)]
                            else:
                                src = xT_ch[(d, 2)][:, (kc - 2) * 512:(kc - 1) * 512]
                            nc.scalar.activation(
                                t8[:, half:half + 1, :],
                                src.rearrange("p (o f) -> p o f", o=1),
                                AF.Copy)
                        xp8_ch[(dp, kc)] = t8
                wkb_all = {}
                wqb_all = {}

                def load_kq_weights(h):
                    # head 0 rides the gpsimd queue behind wvb0; later heads
                    # keep everything on sync so attention's engines stay
                    # DMA-free.
                    eng_k = nc.gpsimd if h == 0 else nc.sync
                    eng_q = nc.gpsimd if h == 0 else nc.sync
                    wkb, wqb = [], []
                    for ct in range(2):
                        c0 = h * DK + ct * 128
                        for dp in range(D // 256):
                            tk = px.tile([128, 2, 128], FP8E4, tag="wkb", bufs=12)
                            tq = px.tile([128, 2, 128], FP8E4, tag="wqb", bufs=12)
                            for half in range(2):
                                d = dp * 2 + half
                                eng_k.dma_start(out=tk[:, half:half + 1, :],
                                                in_=wk[d * 128:(d + 1) * 128,
                                                       c0:c0 + 128])
                                eng_q.dma_start(out=tq[:, half:half + 1, :],
                                                in_=wq[d * 128:(d + 1) * 128,
                                                       c0:c0 + 128])
                            wkb.append(tk)
                            wqb.append(tq)
                    wkb_all[h] = wkb
                    wqb_all[h] = wqb

                load_kq_weights(0)

                xq8_ch = {}
                for dp in range(D // 256):
                    for qc in range(QH // 512):
                        t8 = px.tile([128, 2, 512], FP8E4, tag="xq8", bufs=8,
                                     name="xq8_c")
                        src = bass.AP(tensor=xq8.tensor,
                                      offset=dp * 256 * QH + qc * 512,
                                      ap=[[QH, 128], [128 * QH, 2], [1, 512]])
                        nc.gpsimd.dma_start(out=t8, in_=src)
                        xq8_ch[(dp, qc)] = t8

                for h in range(H):
                    if pending_tail[0] is not None:
                        pending_tail[0]()
                        pending_tail[0] = None
                    v0 = h * DV
                    wvb = wvb_all.pop(h)
                    if h == 0:
                        bvc = bvc0
                    else:
                        bvc = px.tile([128, DV], F32, tag="bvc", bufs=2)
                        nc.gpsimd.dma_start(out=bvc,
                                            in_=_bcast_ap(bv, 128, v0, DV))
                    # V projection -> resident V_h tiles (bf16, key-major)
                    V_h = []
                    for kt in range(S // 128):
                        kc = kt // 4
                        off = ((kt % 4) if kc < 2 else (kt - 8)) * 128
                        ps = ps_mm.tile([128, 512], F32, tag="mm")
                        for d in range(D // 128):
                            nc.tensor.matmul(
                                ps, xT_ch[(d, min(kc, 2))][:, off:off + 128],
                                wvb[d], start=(d == 0), stop=(d == D // 128 - 1))
                        ev = pa.tile([128, DV], BF16, tag="vt", bufs=20)
                        nc.vector.scalar_tensor_tensor(ev, ps,
                                                       mcol[:, kt:kt + 1], bvc,
                                                       op0=AL.mult, op1=AL.add)
                        V_h.append(ev)
                    # K projection -> resident KT8h (fp8, [feat, 2ct, keys])
                    wkb = wkb_all.pop(h)
                    KT8h = pa.tile([128, 2, S], FP8E4, tag="kt", bufs=2)
                    for ct in range(2):
                        for kc in range(S // 512):
                            ps = ps_mm.tile([128, 512], F32, tag="mm")
                            for dp in range(D // 256):
                                nc.tensor.matmul(ps, wkb[ct * 4 + dp], xp8_ch[(dp, kc)],
                                                 start=(dp == 0),
                                                 stop=(dp == D // 256 - 1),
                                                 perf_mode=DR)
                            nc.vector.tensor_scalar(
                                KT8h[:, ct:ct + 1, kc * 512:(kc + 1) * 512], ps,
                                bk16_cols[:, h * 2 + ct:h * 2 + ct + 1],
                                0.0625, op0=AL.add, op1=AL.mult)
                    # Q projection (this core's query half) -> resident QT8h
                    wqb = wqb_all.pop(h)
                    QT8h = pa.tile([128, 2, QH], FP8E4, tag="qt", bufs=2)
                    for ct in range(2):
                        for qc in range(QH // 512):
                            ps = ps_mm.tile([128, 512], F32, tag="mm")
                            for dp in range(D // 256):
                                nc.tensor.matmul(ps, wqb[ct * 4 + dp],
                                                 xq8_ch[(dp, qc)],
                                                 start=(dp == 0),
                                                 stop=(dp == D // 256 - 1),
                                                 perf_mode=DR)
                            nc.vector.tensor_scalar(
                                QT8h[:, ct:ct + 1, qc * 512:(qc + 1) * 512], ps,
                                bq16_cols[:, h * 2 + ct:h * 2 + ct + 1],
                                0.0625, op0=AL.add, op1=AL.mult)

                    # prefetch next head's weights while this head's attention runs
                    if h + 1 < H:
                        wvb_n = []
                        for d in range(D // 128):
                            tw = px.tile([128, DV], BF16, tag="wvb", bufs=16)
                            nc.sync.dma_start(out=tw,
                                              in_=wv[d * 128:(d + 1) * 128,
                                                     (h + 1) * DV:(h + 2) * DV])
                            wvb_n.append(tw)
                        wvb_all[h + 1] = wvb_n
                        load_kq_weights(h + 1)
                    else:
                        # x / weights are dead: free their region and stream
                        # W_o into it while head 7's attention runs. All on
                        # sync: scalar carries exp, gpsimd the odd acc chain.
                        px_cm.__exit__(None, None, None)
                        pwo_cm = tc.tile_pool(name="pwo", bufs=2)
                        pwo = pwo_cm.__enter__()
                        # b_o halves broadcast as rows; needed only by phase C
                        nc.sync.dma_start(out=bo_ah,
                                          in_=_bcast_ap(boh, 128, 0, D))
                        nc.sync.dma_start(out=bo_gh,
                                          in_=_bcast_ap(boh, 128, D, D))
                        for ct in range(H * DK // 128):
                            for jc in range(DO // 512):
                                t5 = pwo.tile([128, 512], BF16, tag="wo5", bufs=64,
                                              name="wo5")
                                nc.sync.dma_start(
                                    out=t5, in_=wo[ct * 128:(ct + 1) * 128,
                                                   jc * 512:(jc + 1) * 512])
                                wo5[(ct, jc)] = t5

                    attention(h, V_h, KT8h, QT8h)

                if pending_tail[0] is not None:
                    pending_tail[0]()
                    pending_tail[0] = None

                ps_dn_cm.__exit__(None, None, None)
                ps_ot_cm.__exit__(None, None, None)
                ps_mm_cm.__exit__(None, None, None)

                # ------------- Phase C: out = GLU(G @ W_o + b_o) -------------
                with tc.tile_pool(name="ps_y", bufs=8, space="PSUM") as ps_y:
                    for qb in range(QH // 128):
                        ys = [ps_y.tile([128, 512], F32, tag="y", name=f"y{_jc}")
                              for _jc in range(4)]
                        for ct in range(H * DK // 128):
                            for jc in range(4):
                                nc.tensor.matmul(ys[jc],
                                                 Gch[ct][:, qb * 128:(qb + 1) * 128],
                                                 wo5[(ct, jc)],
                                                 start=(ct == 0),
                                                 stop=(ct == H * DK // 128 - 1))
                        for jc2 in range(2):
                            y1b = pa.tile([128, 512], F32, tag="an", bufs=2,
                                          name="y1b")
                            nc.vector.scalar_tensor_tensor(
                                y1b, ys[jc2], 0.5,
                                bo_ah[:, jc2 * 512:(jc2 + 1) * 512],
                                op0=AL.mult, op1=AL.add)
                            gs = pa.tile([128, 512], F32, tag="gn", bufs=2,
                                         name="gs")
                            nc.vector.scalar_tensor_tensor(
                                gs, ys[2 + jc2], 0.5,
                                bo_gh[:, jc2 * 512:(jc2 + 1) * 512],
                                op0=AL.mult, op1=AL.add)
                            tg2 = pa.tile([128, 512], F32, tag="tg", bufs=2,
                                          name="tg2")
                            nc.scalar.activation(tg2, gs, AF.Tanh)
                            oc = pa.tile([128, 512], F32, tag="bc", bufs=2,
                                         name="oc")
                            nc.vector.scalar_tensor_tensor(oc, tg2, 1.0, y1b,
                                                           op0=AL.add, op1=AL.mult)
                            nc.sync.dma_start(
                                out=out[qb * 128:(qb + 1) * 128,
                                        jc2 * 512:(jc2 + 1) * 512], in_=oc)
                pwo_cm.__exit__(None, None, None)

    nc.compile()
    return nc


_NC = None


def _make_in_maps(inputs):
    x = np.asarray(inputs["x"], dtype=np.float32)
    mask = np.asarray(inputs["mask"])
    bf = ml_dtypes.bfloat16
    f8 = ml_dtypes.float8_e4m3
    W_v = np.ascontiguousarray(np.asarray(inputs["W_v"], dtype=np.float32).astype(bf))
    W_o = np.ascontiguousarray(np.asarray(inputs["W_o"], dtype=np.float32).astype(bf))
    b_q16 = np.ascontiguousarray(16.0 * np.asarray(inputs["b_q"], dtype=np.float32))
    b_k16 = np.ascontiguousarray(16.0 * np.asarray(inputs["b_k"], dtype=np.float32))
    b_v = np.ascontiguousarray(np.asarray(inputs["b_v"], dtype=np.float32))
    b_vh = np.ascontiguousarray(0.5 * b_v)
    b_oh = np.ascontiguousarray(0.5 * np.asarray(inputs["b_o"], dtype=np.float32))
    W_q8 = np.ascontiguousarray((np.asarray(inputs["W_q"], dtype=np.float32) * 16.0).astype(f8))
    W_k8 = np.ascontiguousarray((np.asarray(inputs["W_k"], dtype=np.float32) * 16.0).astype(f8))
    in_maps = []
    for core in range(N_CORES):
        b, g = core // 2, core % 2
        xT_f = np.ascontiguousarray(x[b].T)
        maskf = np.ascontiguousarray(mask[b].astype(np.float32))
        in_maps.append({
            "xT": np.ascontiguousarray(xT_f.astype(bf)),
            "xq8": np.ascontiguousarray(
                xT_f[:, g * QH:(g + 1) * QH].astype(f8)),
            "wq": W_q8, "wk": W_k8, "wv": W_v, "wo": W_o,
            "bq16": b_q16, "bk16": b_k16, "bv": b_v, "bvh": b_vh, "boh": b_oh,
            "maskf": maskf,
            "maskh": np.ascontiguousarray(0.5 * maskf),
        })
    return in_maps


def kernel(**inputs):
    global _NC
    if _NC is None:
        _NC = _build()
    in_maps = _make_in_maps(inputs)
    res = run_bass_kernel_spmd(_NC, in_maps, core_ids=list(range(N_CORES)))
    B = 4
    out = np.empty((B, S, D), dtype=np.float32)
    for core in range(N_CORES):
        b, g = core // 2, core % 2
        out[b, g * QH:(g + 1) * QH, :] = res.results[core]["out"]
    return out


# revision 53
# speedup vs baseline: 1.0042x; 1.0042x over previous
"""Trainium2 Bass kernel for an 8-head GLU multi-head self-attention block.

Shapes (hardcoded from the problem spec):
  x [4, 2048, 1024], mask [4, 2048] (int32),
  W_q/W_k [1024, 2048], W_v [1024, 4096], W_o [2048, 2048],
  b_q/b_k [2048], b_v [4096], b_o [2048]  ->  out [4, 2048, 1024] f32.

Sharding: 8 cores = 4 batches x 2 query-halves. Each core computes K/V
projections for its full batch (duplicated within the pair - keeps the
program collective-free), Q projection + attention + output projection +
GLUs for its 1024-query half, all 8 heads.

Single merged pass: per head, the V/K/Q projections write straight into
SBUF (no DRAM spill), then attention for that head runs immediately;
GLU'd per-head outputs accumulate into resident SBUF tiles that phase C
consumes directly. x stays SBUF-resident (bf16 for V-proj, fp8 for K/Q
DoubleRow matmuls); W_o streams in while head 7's attention runs. The
only HBM traffic is the original inputs plus the output. All bias/mask
scaling variants (0.5x, 16x) are prepared on host so the kernel's const
loads are four single-descriptor strided DMAs.
"""

import sys
import numpy as np

for _p in ("/opt/trn_rl_repo", "/root/.axon_site/_ro/trn_rl_repo"):
    if _p not in sys.path:
        sys.path.insert(0, _p)

import ml_dtypes
import concourse.bass as bass
import concourse.mybir as mybir
import concourse.tile as tile
from concourse import bacc
from concourse.bass_utils import run_bass_kernel_spmd

F32 = mybir.dt.float32
F32R = mybir.dt.float32r
BF16 = mybir.dt.bfloat16
FP8E4 = mybir.dt.float8e4
DR = mybir.MatmulPerfMode.DoubleRow
AL = mybir.AluOpType
AF = mybir.ActivationFunctionType

N_CORES = 8
S = 2048          # sequence length
D = 1024          # d_model
H = 8             # heads
DK = 256          # per-head q/k dim
DV = 512          # per-head v dim (GLU-doubled)
DO = 2048         # output-projection dim (GLU-doubled)
QH = S // 2       # queries per core


def _bcast_ap(vec_ap, parts, offset, n):
    """AP reading vec[offset:offset+n] broadcast across `parts` partitions."""
    return bass.AP(tensor=vec_ap.tensor, offset=offset, ap=[[0, parts], [1, n]])


def _col_ap(vec_ap, cols):
    """AP viewing a [128*cols] vector as a [128, cols] column tile."""
    return bass.AP(tensor=vec_ap.tensor, offset=0, ap=[[1, 128], [128, cols]])


def _build():
    nc = bacc.Bacc("TRN2", target_bir_lowering=False, debug=False,
                   num_devices=N_CORES)

    xT = nc.dram_tensor("xT", [D, S], BF16, kind="ExternalInput").ap()
    xq8 = nc.dram_tensor("xq8", [D, QH], FP8E4, kind="ExternalInput").ap()
    wq = nc.dram_tensor("wq", [D, H * DK], FP8E4, kind="ExternalInput").ap()
    wk = nc.dram_tensor("wk", [D, H * DK], FP8E4, kind="ExternalInput").ap()
    wv = nc.dram_tensor("wv", [D, H * DV], BF16, kind="ExternalInput").ap()
    wo = nc.dram_tensor("wo", [H * DK, DO], BF16, kind="ExternalInput").ap()
    bq16 = nc.dram_tensor("bq16", [H * DK], F32, kind="ExternalInput").ap()
    bk16 = nc.dram_tensor("bk16", [H * DK], F32, kind="ExternalInput").ap()
    bv = nc.dram_tensor("bv", [H * DV], F32, kind="ExternalInput").ap()
    bvh = nc.dram_tensor("bvh", [H * DV], F32, kind="ExternalInput").ap()
    boh = nc.dram_tensor("boh", [DO], F32, kind="ExternalInput").ap()
    maskf = nc.dram_tensor("maskf", [S], F32, kind="ExternalInput").ap()
    maskh = nc.dram_tensor("maskh", [S], F32, kind="ExternalInput").ap()
    # out is [queries, d_model]: phase C computes G^T-stationary @ wo-moving
    # with queries on the PSUM partition dim, so no output transpose.
    out = nc.dram_tensor("out", [QH, D], F32, kind="ExternalOutput").ap()

    with tile.TileContext(nc) as tc:
        with tc.tile_pool(name="consts", bufs=1) as consts:
            # Single-descriptor strided loads on the (empty) scalar queue.
            mcol = consts.tile([128, S // 128], F32)
            nc.scalar.dma_start(out=mcol, in_=_col_ap(maskf, S // 128))
            mhalf = consts.tile([128, S // 128], F32)
            nc.scalar.dma_start(out=mhalf, in_=_col_ap(maskh, S // 128))
            # head-0 value-bias broadcast next (first V evac needs it early)
            bvc0 = consts.tile([128, DV], F32, name="bvc0")
            nc.scalar.dma_start(out=bvc0, in_=_bcast_ap(bv, 128, 0, DV))
            bq16_cols = consts.tile([128, H * DK // 128], F32)
            nc.scalar.dma_start(out=bq16_cols, in_=_col_ap(bq16, H * DK // 128))
            bk16_cols = consts.tile([128, H * DK // 128], F32)
            nc.scalar.dma_start(out=bk16_cols, in_=_col_ap(bk16, H * DK // 128))
            # Rows/columns of ones for the denominator broadcast matmuls.
            ones_f = consts.tile([1, 128], F32)
            nc.vector.memset(ones_f, 1.0)
            ones1 = consts.tile([1, 128], F32R)
            nc.vector.tensor_copy(ones1, ones_f)
            ones_c = consts.tile([128, 1], F32)
            nc.vector.memset(ones_c, 1.0)
            ones128 = consts.tile([128, 1], F32R)
            nc.vector.tensor_copy(ones128, ones_c)

            with tc.tile_pool(name="pa", bufs=2) as pa:
                ps_mm_cm = tc.tile_pool(name="ps_mm", bufs=3, space="PSUM")
                ps_ot_cm = tc.tile_pool(name="ps_ot", bufs=4, space="PSUM")
                ps_dn_cm = tc.tile_pool(name="ps_dn", bufs=1, space="PSUM")
                ps_mm = ps_mm_cm.__enter__()
                ps_ot = ps_ot_cm.__enter__()
                ps_dn = ps_dn_cm.__enter__()

                # Per-head GLU'd attention outputs, resident until phase C.
                Gch = [pa.tile([128, QH], BF16, tag="gch", bufs=16, name="gch")
                       for _ in range(H * DK // 128)]

                pending_tail = [None]
                wo5 = {}

                def attention(h, V_h, KT8h, QT8h):
                    for qc in range(QH // 512):
                        q0 = qc * 512
                        if pending_tail[0] is not None:
                            pending_tail[0]()
                            pending_tail[0] = None
                        ET = []
                        acc = None
                        for kt in range(S // 128):
                            st = ps_mm.tile([128, 512], F32, tag="mm")
                            nc.tensor.matmul(st, KT8h[:, :, kt * 128:(kt + 1) * 128],
                                             QT8h[:, :, q0:q0 + 512],
                                             start=True, stop=True, perf_mode=DR)
                            e = pa.tile([128, 512], BF16, tag="et", bufs=17)
                            nc.scalar.activation(e, st, AF.Exp, scale=0.0625)
                            ET.append(e)
                            # masked-exp running sum on DVE (ping-pong)
                            nacc = pa.tile([128, 512], F32R, tag="acc", bufs=2,
                                           name="acc")
                            if acc is None:
                                nc.vector.tensor_scalar(nacc, e, mcol[:, kt:kt + 1],
                                                        None, op0=AL.mult)
                            else:
                                nc.vector.scalar_tensor_tensor(
                                    nacc, e, mcol[:, kt:kt + 1], acc,
                                    op0=AL.mult, op1=AL.add)
                            acc = nacc
                        ots = [ps_ot.tile([128, 512], F32, tag="ot", name=f"ot{_i}")
                               for _i in range(4)]
                        for kt in range(S // 128):
                            for dvt in range(4):
                                nc.tensor.matmul(ots[dvt],
                                                 V_h[kt][:, dvt * 128:(dvt + 1) * 128],
                                                 ET[kt], start=(kt == 0),
                                                 stop=(kt == S // 128 - 1))
                        den = ps_dn.tile([1, 512], F32, tag="den")
                        nc.tensor.matmul(den, ones128, acc, start=True, stop=True)
                        dsb = pa.tile([1, 512], F32R, tag="dsb", bufs=2)
                        nc.vector.tensor_copy(dsb, den)
                        bcp = ps_dn.tile([128, 512], F32, tag="den")
                        nc.tensor.matmul(bcp, ones1, dsb, start=True, stop=True)
                        bc = pa.tile([128, 512], F32, tag="bc", bufs=2)
                        nc.vector.reciprocal_approx_fast(bc, bcp)

                        def _tail(h=h, q0=q0, ots=ots, bc=bc):
                            # all four ots reads first so the PSUM bank ring
                            # frees before the ACT-gated Gch writes; the GLU
                            # sigmoid identity's 0.5 rides the a-half op here
                            ans, gns = [], []
                            for c2 in range(2):
                                an = pa.tile([128, 512], F32, tag="an", bufs=2,
                                             name="an")
                                nc.vector.scalar_tensor_tensor(
                                    an, ots[c2], 0.5, bc,
                                    op0=AL.mult, op1=AL.mult)
                                gn = pa.tile([128, 512], F32, tag="gn", bufs=2,
                                             name="gn")
                                nc.vector.tensor_tensor(gn, ots[2 + c2], bc, AL.mult)
                                ans.append(an)
                                gns.append(gn)
                            for c2 in range(2):
                                tg = pa.tile([128, 512], F32, tag="tg", bufs=2,
                                             name="tg")
                                nc.scalar.activation(tg, gns[c2], AF.Tanh, scale=0.5)
                                nc.vector.scalar_tensor_tensor(
                                    Gch[2 * h + c2][:, q0:q0 + 512], tg, 1.0,
                                    ans[c2], op0=AL.add, op1=AL.mult)
                        pending_tail[0] = _tail

                # -------- merged projection + attention loop over heads --------
                px_cm = tc.tile_pool(name="px", bufs=2)
                px = px_cm.__enter__()

                # x resident: bf16 chunks (V-proj stationary) on sync, fp8
                # (K/Q DoubleRow moving) on gpsimd. Head-0 V weights go on
                # the scalar queue so they stream concurrently with xT.
                # Interleave xT-kc0 with head-0 V weights on sync so the first
                # V psum group can start ASAP; the scalar queue carries ONLY
                # the const loads (ACT owns the scalar engine once attention
                # starts — DMA descriptors there would head-of-line block exp).
                wvb_all = {}
                xT_ch = {}
                for d in range(D // 128):
                    tw = px.tile([128, DV], BF16, tag="wvb", bufs=16, name="wvb")
                    nc.gpsimd.dma_start(out=tw, in_=wv[d * 128:(d + 1) * 128, 0:DV])
                    wvb_all.setdefault(0, []).append(tw)
                    t = px.tile([128, 512], BF16, tag="xT_sb", bufs=16, name="xT_c")
                    nc.sync.dma_start(out=t, in_=xT[d * 128:(d + 1) * 128, 0:512])
                    xT_ch[(d, 0)] = t
                bo_ah = consts.tile([128, D], F32, name="bo_ah")
                bo_gh = consts.tile([128, D], F32, name="bo_gh")
                for d in range(D // 128):
                    t = px.tile([128, 512], BF16, tag="xT_sb", bufs=16,
                                name="xT_c")
                    nc.sync.dma_start(out=t, in_=xT[d * 128:(d + 1) * 128,
                                                   512:1024])
                    xT_ch[(d, 1)] = t
                for d in range(D // 128):
                    t = px.tile([128, 1024], BF16, tag="xT_sb2", bufs=8,
                                name="xT_c2")
                    nc.sync.dma_start(out=t, in_=xT[d * 128:(d + 1) * 128,
                                                   1024:2048])
                    xT_ch[(d, 2)] = t
                    xT_ch[(d, 3)] = t
                # fp8 x for the DoubleRow K matmuls is cast on-device from the
                # resident bf16 xT by the (idle until attention) ACT engine —
                # saves 2MB of HBM reads during the bandwidth-bound start.
                xp8_ch = {}
                for kc in range(S // 512):
                    for dp in range(D // 256):
                        t8 = px.tile([128, 2, 512], FP8E4, tag="xp8", bufs=16,
                                     name="xp8_c")
                        for half in range(2):
                            d = dp * 2 + half
                            if kc < 2:
                                src = xT_ch[(d, kc)]
                            else:
                                src = xT_ch[(d, 2)][:, (kc - 2) * 512:(kc - 1) * 512]
                            nc.scalar.activation(
                                t8[:, half:half + 1, :],
                                src.rearrange("p (o f) -> p o f", o=1),
                                AF.Copy)
                        xp8_ch[(dp, kc)] = t8
                wkb_all = {}
                wqb_all = {}

                def load_kq_weights(h):
                    # head 0 rides the gpsimd queue behind wvb0; later heads
                    # keep everything on sync so attention's engines stay
                    # DMA-free.
                    eng_k = nc.gpsimd if h == 0 else nc.sync
                    eng_q = nc.gpsimd if h == 0 else nc.sync
                    wkb, wqb = [], []
                    for ct in range(2):
                        c0 = h * DK + ct * 128
                        for dp in range(D // 256):
                            tk = px.tile([128, 2, 128], FP8E4, tag="wkb", bufs=12)
                            tq = px.tile([128, 2, 128], FP8E4, tag="wqb", bufs=12)
                            for half in range(2):
                                d = dp * 2 + half
                                eng_k.dma_start(out=tk[:, half:half + 1, :],
                                                in_=wk[d * 128:(d + 1) * 128,
                                                       c0:c0 + 128])
                                eng_q.dma_start(out=tq[:, half:half + 1, :],
                                                in_=wq[d * 128:(d + 1) * 128,
                                                       c0:c0 + 128])
                            wkb.append(tk)
                            wqb.append(tq)
                    wkb_all[h] = wkb
                    wqb_all[h] = wqb

                load_kq_weights(0)

                xq8_ch = {}
                for dp in range(D // 256):
                    for qc in range(QH // 512):
                        t8 = px.tile([128, 2, 512], FP8E4, tag="xq8", bufs=8,
                                     name="xq8_c")
                        src = bass.AP(tensor=xq8.tensor,
                                      offset=dp * 256 * QH + qc * 512,
                                      ap=[[QH, 128], [128 * QH, 2], [1, 512]])
                        nc.gpsimd.dma_start(out=t8, in_=src)
                        xq8_ch[(dp, qc)] = t8

                for h in range(H):
                    if pending_tail[0] is not None:
                        pending_tail[0]()
                        pending_tail[0] = None
                    v0 = h * DV
                    wvb = wvb_all.pop(h)
                    if h == 0:
                        bvc = bvc0
                    else:
                        bvc = px.tile([128, DV], F32, tag="bvc", bufs=2)
                        nc.gpsimd.dma_start(out=bvc,
                                            in_=_bcast_ap(bv, 128, v0, DV))
                    # V projection -> resident V_h tiles (bf16, key-major)
                    V_h = []
                    for kt in range(S // 128):
                        kc = kt // 4
                        off = ((kt % 4) if kc < 2 else (kt - 8)) * 128
                        ps = ps_mm.tile([128, 512], F32, tag="mm")
                        for d in range(D // 128):
                            nc.tensor.matmul(
                                ps, xT_ch[(d, min(kc, 2))][:, off:off + 128],
                                wvb[d], start=(d == 0), stop=(d == D // 128 - 1))
                        ev = pa.tile([128, DV], BF16, tag="vt", bufs=20)
                        nc.vector.scalar_tensor_tensor(ev, ps,
                                                       mcol[:, kt:kt + 1], bvc,
                                                       op0=AL.mult, op1=AL.add)
                        V_h.append(ev)
                    # K projection -> resident KT8h (fp8, [feat, 2ct, keys])
                    wkb = wkb_all.pop(h)
                    KT8h = pa.tile([128, 2, S], FP8E4, tag="kt", bufs=2)
                    for ct in range(2):
                        for kc in range(S // 512):
                            ps = ps_mm.tile([128, 512], F32, tag="mm")
                            for dp in range(D // 256):
                                nc.tensor.matmul(ps, wkb[ct * 4 + dp], xp8_ch[(dp, kc)],
                                                 start=(dp == 0),
                                                 stop=(dp == D // 256 - 1),
                                                 perf_mode=DR)
                            nc.vector.tensor_scalar(
                                KT8h[:, ct:ct + 1, kc * 512:(kc + 1) * 512], ps,
                                bk16_cols[:, h * 2 + ct:h * 2 + ct + 1],
                                0.0625, op0=AL.add, op1=AL.mult)
                    # Q projection (this core's query half) -> resident QT8h
                    wqb = wqb_all.pop(h)
                    QT8h = pa.tile([128, 2, QH], FP8E4, tag="qt", bufs=2)
                    for ct in range(2):
                        for qc in range(QH // 512):
                            ps = ps_mm.tile([128, 512], F32, tag="mm")
                            for dp in range(D // 256):
                                nc.tensor.matmul(ps, wqb[ct * 4 + dp],
                                                 xq8_ch[(dp, qc)],
                                                 start=(dp == 0),
                                                 stop=(dp == D // 256 - 1),
                                                 perf_mode=DR)
                            nc.vector.tensor_scalar(
                                QT8h[:, ct:ct + 1, qc * 512:(qc + 1) * 512], ps,
                                bq16_cols[:, h * 2 + ct:h * 2 + ct + 1],
                                0.0625, op0=AL.add, op1=AL.mult)

                    # prefetch next head's weights while this head's attention runs
                    if h + 1 < H:
                        wvb_n = []
                        for d in range(D // 128):
                            tw = px.tile([128, DV], BF16, tag="wvb", bufs=16)
                            nc.sync.dma_start(out=tw,
                                              in_=wv[d * 128:(d + 1) * 128,
                                                     (h + 1) * DV:(h + 2) * DV])
                            wvb_n.append(tw)
                        wvb_all[h + 1] = wvb_n
                        load_kq_weights(h + 1)
                    else:
                        # x / weights are dead: free their region and stream
                        # W_o into it while head 7's attention runs. All on
                        # sync: scalar carries exp, gpsimd the odd acc chain.
                        px_cm.__exit__(None, None, None)
                        pwo_cm = tc.tile_pool(name="pwo", bufs=2)
                        pwo = pwo_cm.__enter__()
                        # b_o halves broadcast as rows; needed only by phase C
                        nc.sync.dma_start(out=bo_ah,
                                          in_=_bcast_ap(boh, 128, 0, D))
                        nc.sync.dma_start(out=bo_gh,
                                          in_=_bcast_ap(boh, 128, D, D))
                        for ct in range(H * DK // 128):
                            for jc in range(DO // 512):
                                t5 = pwo.tile([128, 512], BF16, tag="wo5", bufs=64,
                                              name="wo5")
                                nc.sync.dma_start(
                                    out=t5, in_=wo[ct * 128:(ct + 1) * 128,
                                                   jc * 512:(jc + 1) * 512])
                                wo5[(ct, jc)] = t5

                    attention(h, V_h, KT8h, QT8h)

                if pending_tail[0] is not None:
                    pending_tail[0]()
                    pending_tail[0] = None

                ps_dn_cm.__exit__(None, None, None)
                ps_ot_cm.__exit__(None, None, None)
                ps_mm_cm.__exit__(None, None, None)

                # ------------- Phase C: out = GLU(G @ W_o + b_o) -------------
                with tc.tile_pool(name="ps_y", bufs=8, space="PSUM") as ps_y:
                    for qb in range(QH // 128):
                        ys = [ps_y.tile([128, 512], F32, tag="y", name=f"y{_jc}")
                              for _jc in range(4)]
                        for ct in range(H * DK // 128):
                            for jc in range(4):
                                nc.tensor.matmul(ys[jc],
                                                 Gch[ct][:, qb * 128:(qb + 1) * 128],
                                                 wo5[(ct, jc)],
                                                 start=(ct == 0),
                                                 stop=(ct == H * DK // 128 - 1))
                        for jc2 in range(2):
                            y1b = pa.tile([128, 512], F32, tag="an", bufs=2,
                                          name="y1b")
                            nc.vector.scalar_tensor_tensor(
                                y1b, ys[jc2], 0.5,
                                bo_ah[:, jc2 * 512:(jc2 + 1) * 512],
                                op0=AL.mult, op1=AL.add)
                            gs = pa.tile([128, 512], F32, tag="gn", bufs=2,
                                         name="gs")
                            nc.vector.scalar_tensor_tensor(
                                gs, ys[2 + jc2], 0.5,
                                bo_gh[:, jc2 * 512:(jc2 + 1) * 512],
                                op0=AL.mult, op1=AL.add)
                            tg2 = pa.tile([128, 512], F32, tag="tg", bufs=2,
                                          name="tg2")
                            nc.scalar.activation(tg2, gs, AF.Tanh)
                            oc = pa.tile([128, 512], F32, tag="bc", bufs=2,
                                         name="oc")
                            nc.vector.scalar_tensor_tensor(oc, tg2, 1.0, y1b,
                                                           op0=AL.add, op1=AL.mult)
                            nc.sync.dma_start(
                                out=out[qb * 128:(qb + 1) * 128,
                                        jc2 * 512:(jc2 + 1) * 512], in_=oc)
                pwo_cm.__exit__(None, None, None)

    nc.compile()
    return nc


_NC = None


def _make_in_maps(inputs):
    x = np.asarray(inputs["x"], dtype=np.float32)
    mask = np.asarray(inputs["mask"])
    bf = ml_dtypes.bfloat16
    f8 = ml_dtypes.float8_e4m3
    W_v = np.ascontiguousarray(np.asarray(inputs["W_v"], dtype=np.float32).astype(bf))
    W_o = np.ascontiguousarray(np.asarray(inputs["W_o"], dtype=np.float32).astype(bf))
    b_q16 = np.ascontiguousarray(16.0 * np.asarray(inputs["b_q"], dtype=np.float32))
    b_k16 = np.ascontiguousarray(16.0 * np.asarray(inputs["b_k"], dtype=np.float32))
    b_v = np.ascontiguousarray(np.asarray(inputs["b_v"], dtype=np.float32))
    b_vh = np.ascontiguousarray(0.5 * b_v)
    b_oh = np.ascontiguousarray(0.5 * np.asarray(inputs["b_o"], dtype=np.float32))
    W_q8 = np.ascontiguousarray((np.asarray(inputs["W_q"], dtype=np.float32) * 16.0).astype(f8))
    W_k8 = np.ascontiguousarray((np.asarray(inputs["W_k"], dtype=np.float32) * 16.0).astype(f8))
    in_maps = []
    for core in range(N_CORES):
        b, g = core // 2, core % 2
        xT_f = np.ascontiguousarray(x[b].T)
        maskf = np.ascontiguousarray(mask[b].astype(np.float32))
        in_maps.append({
            "xT": np.ascontiguousarray(xT_f.astype(bf)),
            "xq8": np.ascontiguousarray(
                xT_f[:, g * QH:(g + 1) * QH].astype(f8)),
            "wq": W_q8, "wk": W_k8, "wv": W_v, "wo": W_o,
            "bq16": b_q16, "bk16": b_k16, "bv": b_v, "bvh": b_vh, "boh": b_oh,
            "maskf": maskf,
            "maskh": np.ascontiguousarray(0.5 * maskf),
        })
    return in_maps


def kernel(**inputs):
    global _NC
    if _NC is None:
        _NC = _build()
    in_maps = _make_in_maps(inputs)
    res = run_bass_kernel_spmd(_NC, in_maps, core_ids=list(range(N_CORES)))
    B = 4
    out = np.empty((B, S, D), dtype=np.float32)
    for core in range(N_CORES):
        b, g = core // 2, core % 2
        out[b, g * QH:(g + 1) * QH, :] = res.results[core]["out"]
    return out


# revision 54
# speedup vs baseline: 1.0073x; 1.0031x over previous
"""Trainium2 Bass kernel for an 8-head GLU multi-head self-attention block.

Shapes (hardcoded from the problem spec):
  x [4, 2048, 1024], mask [4, 2048] (int32),
  W_q/W_k [1024, 2048], W_v [1024, 4096], W_o [2048, 2048],
  b_q/b_k [2048], b_v [4096], b_o [2048]  ->  out [4, 2048, 1024] f32.

Sharding: 8 cores = 4 batches x 2 query-halves. Each core computes K/V
projections for its full batch (duplicated within the pair - keeps the
program collective-free), Q projection + attention + output projection +
GLUs for its 1024-query half, all 8 heads.

Single merged pass: per head, the V/K/Q projections write straight into
SBUF (no DRAM spill), then attention for that head runs immediately;
GLU'd per-head outputs accumulate into resident SBUF tiles that phase C
consumes directly. x stays SBUF-resident (bf16 for V-proj, fp8 for K/Q
DoubleRow matmuls); W_o streams in while head 7's attention runs. The
only HBM traffic is the original inputs plus the output. All bias/mask
scaling variants (0.5x, 16x) are prepared on host so the kernel's const
loads are four single-descriptor strided DMAs.
"""

import sys
import numpy as np

for _p in ("/opt/trn_rl_repo", "/root/.axon_site/_ro/trn_rl_repo"):
    if _p not in sys.path:
        sys.path.insert(0, _p)

import ml_dtypes
import concourse.bass as bass
import concourse.mybir as mybir
import concourse.tile as tile
from concourse import bacc
from concourse.bass_utils import run_bass_kernel_spmd

F32 = mybir.dt.float32
F32R = mybir.dt.float32r
BF16 = mybir.dt.bfloat16
FP8E4 = mybir.dt.float8e4
DR = mybir.MatmulPerfMode.DoubleRow
AL = mybir.AluOpType
AF = mybir.ActivationFunctionType

N_CORES = 8
S = 2048          # sequence length
D = 1024          # d_model
H = 8             # heads
DK = 256          # per-head q/k dim
DV = 512          # per-head v dim (GLU-doubled)
DO = 2048         # output-projection dim (GLU-doubled)
QH = S // 2       # queries per core


def _bcast_ap(vec_ap, parts, offset, n):
    """AP reading vec[offset:offset+n] broadcast across `parts` partitions."""
    return bass.AP(tensor=vec_ap.tensor, offset=offset, ap=[[0, parts], [1, n]])


def _col_ap(vec_ap, cols):
    """AP viewing a [128*cols] vector as a [128, cols] column tile."""
    return bass.AP(tensor=vec_ap.tensor, offset=0, ap=[[1, 128], [128, cols]])


def _build():
    nc = bacc.Bacc("TRN2", target_bir_lowering=False, debug=False,
                   num_devices=N_CORES)

    xT = nc.dram_tensor("xT", [D, S], BF16, kind="ExternalInput").ap()
    xq8 = nc.dram_tensor("xq8", [D, QH], FP8E4, kind="ExternalInput").ap()
    wq = nc.dram_tensor("wq", [D, H * DK], FP8E4, kind="ExternalInput").ap()
    wk = nc.dram_tensor("wk", [D, H * DK], FP8E4, kind="ExternalInput").ap()
    wv = nc.dram_tensor("wv", [D, H * DV], BF16, kind="ExternalInput").ap()
    wo = nc.dram_tensor("wo", [H * DK, DO], BF16, kind="ExternalInput").ap()
    bq16 = nc.dram_tensor("bq16", [H * DK], F32, kind="ExternalInput").ap()
    bk16 = nc.dram_tensor("bk16", [H * DK], F32, kind="ExternalInput").ap()
    bv = nc.dram_tensor("bv", [H * DV], F32, kind="ExternalInput").ap()
    boh = nc.dram_tensor("boh", [DO], F32, kind="ExternalInput").ap()
    maskf = nc.dram_tensor("maskf", [S], F32, kind="ExternalInput").ap()
    # out is [queries, d_model]: phase C computes G^T-stationary @ wo-moving
    # with queries on the PSUM partition dim, so no output transpose.
    out = nc.dram_tensor("out", [QH, D], F32, kind="ExternalOutput").ap()

    with tile.TileContext(nc) as tc:
        with tc.tile_pool(name="consts", bufs=1) as consts:
            # Single-descriptor strided loads on the (empty) scalar queue.
            mcol = consts.tile([128, S // 128], F32)
            nc.scalar.dma_start(out=mcol, in_=_col_ap(maskf, S // 128))
            # head-0 value-bias broadcast next (first V evac needs it early)
            bvc0 = consts.tile([128, DV], F32, name="bvc0")
            nc.scalar.dma_start(out=bvc0, in_=_bcast_ap(bv, 128, 0, DV))
            bq16_cols = consts.tile([128, H * DK // 128], F32)
            nc.scalar.dma_start(out=bq16_cols, in_=_col_ap(bq16, H * DK // 128))
            bk16_cols = consts.tile([128, H * DK // 128], F32)
            nc.scalar.dma_start(out=bk16_cols, in_=_col_ap(bk16, H * DK // 128))
            # Rows/columns of ones for the denominator broadcast matmuls.
            ones_f = consts.tile([1, 128], F32)
            nc.vector.memset(ones_f, 1.0)
            ones1 = consts.tile([1, 128], F32R)
            nc.vector.tensor_copy(ones1, ones_f)
            ones_c = consts.tile([128, 1], F32)
            nc.vector.memset(ones_c, 1.0)
            ones128 = consts.tile([128, 1], F32R)
            nc.vector.tensor_copy(ones128, ones_c)

            with tc.tile_pool(name="pa", bufs=2) as pa:
                ps_mm_cm = tc.tile_pool(name="ps_mm", bufs=3, space="PSUM")
                ps_ot_cm = tc.tile_pool(name="ps_ot", bufs=4, space="PSUM")
                ps_dn_cm = tc.tile_pool(name="ps_dn", bufs=1, space="PSUM")
                ps_mm = ps_mm_cm.__enter__()
                ps_ot = ps_ot_cm.__enter__()
                ps_dn = ps_dn_cm.__enter__()

                # Per-head GLU'd attention outputs, resident until phase C.
                Gch = [pa.tile([128, QH], BF16, tag="gch", bufs=16, name="gch")
                       for _ in range(H * DK // 128)]

                pending_tail = [None]
                wo5 = {}

                def attention(h, V_h, KT8h, QT8h):
                    for qc in range(QH // 512):
                        q0 = qc * 512
                        if pending_tail[0] is not None:
                            pending_tail[0]()
                            pending_tail[0] = None
                        ET = []
                        acc = None
                        for kt in range(S // 128):
                            st = ps_mm.tile([128, 512], F32, tag="mm")
                            nc.tensor.matmul(st, KT8h[:, :, kt * 128:(kt + 1) * 128],
                                             QT8h[:, :, q0:q0 + 512],
                                             start=True, stop=True, perf_mode=DR)
                            e = pa.tile([128, 512], BF16, tag="et", bufs=17)
                            nc.scalar.activation(e, st, AF.Exp, scale=0.0625)
                            ET.append(e)
                            # masked-exp running sum on DVE (ping-pong)
                            nacc = pa.tile([128, 512], F32R, tag="acc", bufs=2,
                                           name="acc")
                            if acc is None:
                                nc.vector.tensor_scalar(nacc, e, mcol[:, kt:kt + 1],
                                                        None, op0=AL.mult)
                            else:
                                nc.vector.scalar_tensor_tensor(
                                    nacc, e, mcol[:, kt:kt + 1], acc,
                                    op0=AL.mult, op1=AL.add)
                            acc = nacc
                        ots = [ps_ot.tile([128, 512], F32, tag="ot", name=f"ot{_i}")
                               for _i in range(4)]
                        for kt in range(S // 128):
                            for dvt in range(4):
                                nc.tensor.matmul(ots[dvt],
                                                 V_h[kt][:, dvt * 128:(dvt + 1) * 128],
                                                 ET[kt], start=(kt == 0),
                                                 stop=(kt == S // 128 - 1))
                        den = ps_dn.tile([1, 512], F32, tag="den")
                        nc.tensor.matmul(den, ones128, acc, start=True, stop=True)
                        dsb = pa.tile([1, 512], F32R, tag="dsb", bufs=2)
                        nc.vector.tensor_copy(dsb, den)
                        bcp = ps_dn.tile([128, 512], F32, tag="den")
                        nc.tensor.matmul(bcp, ones1, dsb, start=True, stop=True)
                        bc = pa.tile([128, 512], F32, tag="bc", bufs=2)
                        nc.vector.reciprocal_approx_fast(bc, bcp)

                        def _tail(h=h, q0=q0, ots=ots, bc=bc):
                            # all four ots reads first so the PSUM bank ring
                            # frees before the ACT-gated Gch writes; the GLU
                            # sigmoid identity's 0.5 rides the a-half op here
                            ans, gns = [], []
                            for c2 in range(2):
                                an = pa.tile([128, 512], F32, tag="an", bufs=2,
                                             name="an")
                                nc.vector.scalar_tensor_tensor(
                                    an, ots[c2], 0.5, bc,
                                    op0=AL.mult, op1=AL.mult)
                                gn = pa.tile([128, 512], F32, tag="gn", bufs=2,
                                             name="gn")
                                nc.vector.tensor_tensor(gn, ots[2 + c2], bc, AL.mult)
                                ans.append(an)
                                gns.append(gn)
                            for c2 in range(2):
                                tg = pa.tile([128, 512], F32, tag="tg", bufs=2,
                                             name="tg")
                                nc.scalar.activation(tg, gns[c2], AF.Tanh, scale=0.5)
                                nc.vector.scalar_tensor_tensor(
                                    Gch[2 * h + c2][:, q0:q0 + 512], tg, 1.0,
                                    ans[c2], op0=AL.add, op1=AL.mult)
                        pending_tail[0] = _tail

                # -------- merged projection + attention loop over heads --------
                px_cm = tc.tile_pool(name="px", bufs=2)
                px = px_cm.__enter__()

                # x resident: bf16 chunks (V-proj stationary) on sync, fp8
                # (K/Q DoubleRow moving) on gpsimd. Head-0 V weights go on
                # the scalar queue so they stream concurrently with xT.
                # Interleave xT-kc0 with head-0 V weights on sync so the first
                # V psum group can start ASAP; the scalar queue carries ONLY
                # the const loads (ACT owns the scalar engine once attention
                # starts — DMA descriptors there would head-of-line block exp).
                wvb_all = {}
                xT_ch = {}
                for d in range(D // 128):
                    tw = px.tile([128, DV], BF16, tag="wvb", bufs=16, name="wvb")
                    nc.gpsimd.dma_start(out=tw, in_=wv[d * 128:(d + 1) * 128, 0:DV])
                    wvb_all.setdefault(0, []).append(tw)
                    t = px.tile([128, 512], BF16, tag="xT_sb", bufs=16, name="xT_c")
                    nc.sync.dma_start(out=t, in_=xT[d * 128:(d + 1) * 128, 0:512])
                    xT_ch[(d, 0)] = t
                bo_ah = consts.tile([128, D], F32, name="bo_ah")
                bo_gh = consts.tile([128, D], F32, name="bo_gh")
                for d in range(D // 128):
                    t = px.tile([128, 512], BF16, tag="xT_sb", bufs=16,
                                name="xT_c")
                    nc.sync.dma_start(out=t, in_=xT[d * 128:(d + 1) * 128,
                                                   512:1024])
                    xT_ch[(d, 1)] = t
                for d in range(D // 128):
                    t = px.tile([128, 1024], BF16, tag="xT_sb2", bufs=8,
                                name="xT_c2")
                    nc.sync.dma_start(out=t, in_=xT[d * 128:(d + 1) * 128,
                                                   1024:2048])
                    xT_ch[(d, 2)] = t
                    xT_ch[(d, 3)] = t
                # fp8 x for the DoubleRow K matmuls is cast on-device from the
                # resident bf16 xT by the (idle until attention) ACT engine —
                # saves 2MB of HBM reads during the bandwidth-bound start.
                xp8_ch = {}
                for kc in range(S // 512):
                    for dp in range(D // 256):
                        t8 = px.tile([128, 2, 512], FP8E4, tag="xp8", bufs=16,
                                     name="xp8_c")
                        for half in range(2):
                            d = dp * 2 + half
                            if kc < 2:
                                src = xT_ch[(d, kc)]
                            else:
                                src = xT_ch[(d, 2)][:, (kc - 2) * 512:(kc - 1) * 512]
                            nc.scalar.activation(
                                t8[:, half:half + 1, :],
                                src.rearrange("p (o f) -> p o f", o=1),
                                AF.Copy)
                        xp8_ch[(dp, kc)] = t8
                wkb_all = {}
                wqb_all = {}

                def load_kq_weights(h):
                    # head 0 rides the gpsimd queue behind wvb0; later heads
                    # keep everything on sync so attention's engines stay
                    # DMA-free.
                    eng_k = nc.gpsimd if h == 0 else nc.sync
                    eng_q = nc.gpsimd if h == 0 else nc.sync
                    wkb, wqb = [], []
                    for ct in range(2):
                        c0 = h * DK + ct * 128
                        for dp in range(D // 256):
                            tk = px.tile([128, 2, 128], FP8E4, tag="wkb", bufs=12)
                            tq = px.tile([128, 2, 128], FP8E4, tag="wqb", bufs=12)
                            for half in range(2):
                                d = dp * 2 + half
                                eng_k.dma_start(out=tk[:, half:half + 1, :],
                                                in_=wk[d * 128:(d + 1) * 128,
                                                       c0:c0 + 128])
                                eng_q.dma_start(out=tq[:, half:half + 1, :],
                                                in_=wq[d * 128:(d + 1) * 128,
                                                       c0:c0 + 128])
                            wkb.append(tk)
                            wqb.append(tq)
                    wkb_all[h] = wkb
                    wqb_all[h] = wqb

                load_kq_weights(0)

                xq8_ch = {}
                for dp in range(D // 256):
                    for qc in range(QH // 512):
                        t8 = px.tile([128, 2, 512], FP8E4, tag="xq8", bufs=8,
                                     name="xq8_c")
                        src = bass.AP(tensor=xq8.tensor,
                                      offset=dp * 256 * QH + qc * 512,
                                      ap=[[QH, 128], [128 * QH, 2], [1, 512]])
                        nc.gpsimd.dma_start(out=t8, in_=src)
                        xq8_ch[(dp, qc)] = t8

                for h in range(H):
                    if pending_tail[0] is not None:
                        pending_tail[0]()
                        pending_tail[0] = None
                    v0 = h * DV
                    wvb = wvb_all.pop(h)
                    if h == 0:
                        bvc = bvc0
                    else:
                        bvc = px.tile([128, DV], F32, tag="bvc", bufs=2)
                        nc.gpsimd.dma_start(out=bvc,
                                            in_=_bcast_ap(bv, 128, v0, DV))
                    # V projection -> resident V_h tiles (bf16, key-major)
                    V_h = []
                    for kt in range(S // 128):
                        kc = kt // 4
                        off = ((kt % 4) if kc < 2 else (kt - 8)) * 128
                        ps = ps_mm.tile([128, 512], F32, tag="mm")
                        for d in range(D // 128):
                            nc.tensor.matmul(
                                ps, xT_ch[(d, min(kc, 2))][:, off:off + 128],
                                wvb[d], start=(d == 0), stop=(d == D // 128 - 1))
                        ev = pa.tile([128, DV], BF16, tag="vt", bufs=20)
                        nc.vector.scalar_tensor_tensor(ev, ps,
                                                       mcol[:, kt:kt + 1], bvc,
                                                       op0=AL.mult, op1=AL.add)
                        V_h.append(ev)
                    # K projection -> resident KT8h (fp8, [feat, 2ct, keys])
                    wkb = wkb_all.pop(h)
                    KT8h = pa.tile([128, 2, S], FP8E4, tag="kt", bufs=2)
                    for ct in range(2):
                        for kc in range(S // 512):
                            ps = ps_mm.tile([128, 512], F32, tag="mm")
                            for dp in range(D // 256):
                                nc.tensor.matmul(ps, wkb[ct * 4 + dp], xp8_ch[(dp, kc)],
                                                 start=(dp == 0),
                                                 stop=(dp == D // 256 - 1),
                                                 perf_mode=DR)
                            nc.vector.tensor_scalar(
                                KT8h[:, ct:ct + 1, kc * 512:(kc + 1) * 512], ps,
                                bk16_cols[:, h * 2 + ct:h * 2 + ct + 1],
                                0.0625, op0=AL.add, op1=AL.mult)
                    # Q projection (this core's query half) -> resident QT8h
                    wqb = wqb_all.pop(h)
                    QT8h = pa.tile([128, 2, QH], FP8E4, tag="qt", bufs=2)
                    for ct in range(2):
                        for qc in range(QH // 512):
                            ps = ps_mm.tile([128, 512], F32, tag="mm")
                            for dp in range(D // 256):
                                nc.tensor.matmul(ps, wqb[ct * 4 + dp],
                                                 xq8_ch[(dp, qc)],
                                                 start=(dp == 0),
                                                 stop=(dp == D // 256 - 1),
                                                 perf_mode=DR)
                            nc.vector.tensor_scalar(
                                QT8h[:, ct:ct + 1, qc * 512:(qc + 1) * 512], ps,
                                bq16_cols[:, h * 2 + ct:h * 2 + ct + 1],
                                0.0625, op0=AL.add, op1=AL.mult)

                    # prefetch next head's weights while this head's attention runs
                    if h + 1 < H:
                        wvb_n = []
                        for d in range(D // 128):
                            tw = px.tile([128, DV], BF16, tag="wvb", bufs=16)
                            nc.sync.dma_start(out=tw,
                                              in_=wv[d * 128:(d + 1) * 128,
                                                     (h + 1) * DV:(h + 2) * DV])
                            wvb_n.append(tw)
                        wvb_all[h + 1] = wvb_n
                        load_kq_weights(h + 1)
                    else:
                        # x / weights are dead: free their region and stream
                        # W_o into it while head 7's attention runs. All on
                        # sync: scalar carries exp, gpsimd the odd acc chain.
                        px_cm.__exit__(None, None, None)
                        pwo_cm = tc.tile_pool(name="pwo", bufs=2)
                        pwo = pwo_cm.__enter__()
                        # b_o halves broadcast as rows; needed only by phase C
                        nc.sync.dma_start(out=bo_ah,
                                          in_=_bcast_ap(boh, 128, 0, D))
                        nc.sync.dma_start(out=bo_gh,
                                          in_=_bcast_ap(boh, 128, D, D))
                        for ct in range(H * DK // 128):
                            for jc in range(DO // 512):
                                t5 = pwo.tile([128, 512], BF16, tag="wo5", bufs=64,
                                              name="wo5")
                                nc.sync.dma_start(
                                    out=t5, in_=wo[ct * 128:(ct + 1) * 128,
                                                   jc * 512:(jc + 1) * 512])
                                wo5[(ct, jc)] = t5

                    attention(h, V_h, KT8h, QT8h)

                if pending_tail[0] is not None:
                    pending_tail[0]()
                    pending_tail[0] = None

                ps_dn_cm.__exit__(None, None, None)
                ps_ot_cm.__exit__(None, None, None)
                ps_mm_cm.__exit__(None, None, None)

                # ------------- Phase C: out = GLU(G @ W_o + b_o) -------------
                with tc.tile_pool(name="ps_y", bufs=8, space="PSUM") as ps_y:
                    for qb in range(QH // 128):
                        ys = [ps_y.tile([128, 512], F32, tag="y", name=f"y{_jc}")
                              for _jc in range(4)]
                        for ct in range(H * DK // 128):
                            for jc in range(4):
                                nc.tensor.matmul(ys[jc],
                                                 Gch[ct][:, qb * 128:(qb + 1) * 128],
                                                 wo5[(ct, jc)],
                                                 start=(ct == 0),
                                                 stop=(ct == H * DK // 128 - 1))
                        for jc2 in range(2):
                            y1b = pa.tile([128, 512], F32, tag="an", bufs=2,
                                          name="y1b")
                            nc.vector.scalar_tensor_tensor(
                                y1b, ys[jc2], 0.5,
                                bo_ah[:, jc2 * 512:(jc2 + 1) * 512],
                                op0=AL.mult, op1=AL.add)
                            gs = pa.tile([128, 512], F32, tag="gn", bufs=2,
                                         name="gs")
                            nc.vector.scalar_tensor_tensor(
                                gs, ys[2 + jc2], 0.5,
                                bo_gh[:, jc2 * 512:(jc2 + 1) * 512],
                                op0=AL.mult, op1=AL.add)
                            tg2 = pa.tile([128, 512], F32, tag="tg", bufs=2,
                                          name="tg2")
                            nc.scalar.activation(tg2, gs, AF.Tanh)
                            oc = pa.tile([128, 512], F32, tag="bc", bufs=2,
                                         name="oc")
                            nc.vector.scalar_tensor_tensor(oc, tg2, 1.0, y1b,
                                                           op0=AL.add, op1=AL.mult)
                            nc.sync.dma_start(
                                out=out[qb * 128:(qb + 1) * 128,
                                        jc2 * 512:(jc2 + 1) * 512], in_=oc)
                pwo_cm.__exit__(None, None, None)

    nc.compile()
    return nc


_NC = None


def _make_in_maps(inputs):
    x = np.asarray(inputs["x"], dtype=np.float32)
    mask = np.asarray(inputs["mask"])
    bf = ml_dtypes.bfloat16
    f8 = ml_dtypes.float8_e4m3
    W_v = np.ascontiguousarray(np.asarray(inputs["W_v"], dtype=np.float32).astype(bf))
    W_o = np.ascontiguousarray(np.asarray(inputs["W_o"], dtype=np.float32).astype(bf))
    b_q16 = np.ascontiguousarray(16.0 * np.asarray(inputs["b_q"], dtype=np.float32))
    b_k16 = np.ascontiguousarray(16.0 * np.asarray(inputs["b_k"], dtype=np.float32))
    b_v = np.ascontiguousarray(np.asarray(inputs["b_v"], dtype=np.float32))
    b_oh = np.ascontiguousarray(0.5 * np.asarray(inputs["b_o"], dtype=np.float32))
    W_q8 = np.ascontiguousarray((np.asarray(inputs["W_q"], dtype=np.float32) * 16.0).astype(f8))
    W_k8 = np.ascontiguousarray((np.asarray(inputs["W_k"], dtype=np.float32) * 16.0).astype(f8))
    in_maps = []
    for core in range(N_CORES):
        b, g = core // 2, core % 2
        xT_f = np.ascontiguousarray(x[b].T)
        maskf = np.ascontiguousarray(mask[b].astype(np.float32))
        in_maps.append({
            "xT": np.ascontiguousarray(xT_f.astype(bf)),
            "xq8": np.ascontiguousarray(
                xT_f[:, g * QH:(g + 1) * QH].astype(f8)),
            "wq": W_q8, "wk": W_k8, "wv": W_v, "wo": W_o,
            "bq16": b_q16, "bk16": b_k16, "bv": b_v, "boh": b_oh,
            "maskf": maskf,
        })
    return in_maps


def kernel(**inputs):
    global _NC
    if _NC is None:
        _NC = _build()
    in_maps = _make_in_maps(inputs)
    res = run_bass_kernel_spmd(_NC, in_maps, core_ids=list(range(N_CORES)))
    B = 4
    out = np.empty((B, S, D), dtype=np.float32)
    for core in range(N_CORES):
        b, g = core // 2, core % 2
        out[b, g * QH:(g + 1) * QH, :] = res.results[core]["out"]
    return out
